# revision 6
# baseline (speedup 1.0000x reference)
"""Masked mean-pool (NonZeroAvgPool) Trainium2 Bass kernel.

out[b, d] = sum_s (tokens[b,s] != 0) * x[b,s,d] / sum_s (tokens[b,s] != 0)

Full shapes: x [16, 4096, 512] f32, tokens [16, 4096] i32 -> out [16, 512] f32.
Sharding: pure data parallel over batch; 2 batches per core on 8 cores.

Per-core program (shapes [2, 4096, 512] / [2, 4096] -> [2, 512]):
  masked sum via PE: num[1, D] = sum_c valid[:, c].T @ x_tile[:, c, :]
  accumulated in PSUM; count via ones.T @ rowsum(valid); divide on DVE.

Layout (the interesting part). SBUF AXI port k serves a fixed set of 8
partitions; HW traces show port/engine 15 (partitions {92-95, 124-127})
intermittently runs ~15-20% slower than the rest, and every DMA completion
semaphore waits for ALL 16 engines.  So the sequence axis is laid out
RAGGED to statically de-weight port 15:

  partitions   0..123: 33 rows each   (rows p*33 + c, c in [0,33))
  partitions 124..127:  1 row each    (rows 4092 + (p-124))

Port 15 then carries 4*33 + 4*1 = 136 rows vs 264 for every other port
(52%), so even a 2x-slow engine 15 finishes early.  On good days the other
ports pay only +3% bytes.  Chunks c >= 1 use K=124 matmuls (partitions
124..127 never read -> no SBUF padding needed); chunk 0 uses K=128.

x streams in FINE-GRAINED DMAs (4 chunks = 6.1MB.. 4*124*2KB each, 8KB
per-partition descriptors) so each matmul is gated by a small transfer:
the PE trails the stream by ~1 DMA, stays HAM-warm, and the tail after the
last byte is one 1-chunk matmul + divide + store.  The tile framework
round-robins the 8 DMAHW completion-sem lanes with automatic reuse waits.
"""

import os
from contextlib import ExitStack

import numpy as np

import concourse.bacc as bacc
import concourse.bass as bass
import concourse.tile as tile
from concourse import mybir
from concourse.bass_utils import run_bass_kernel_spmd

B, S, D = 16, 4096, 512
NCORES = 8
BPC = B // NCORES   # batches per core = 2
P = 128             # SBUF partitions
PN = 124            # "normal" partitions (full stream)
CN = 33             # chunks per normal partition
CS = 1              # chunks per slow partition (124..127)
assert PN * CN + (P - PN) * CS == S

# Chunks per A-stream dma_start: CN = 4*8 + 1 -> 8 DMAs of 4 chunks plus a
# final 1-chunk DMA, so the last completion sem gates a single matmul.
AG = int(os.environ.get("K_AG", "4"))
X_ENGINE = os.environ.get("K_XENG", "act")  # sync | act | gpsimd

_NC = None


def _a_groups():
    """Chunk ranges for the A (normal-partition) stream: [(c0, c1), ...]."""
    gs = []
    c = 0
    while c + AG <= CN - 1:
        gs.append((c, c + AG))
        c += AG
    while c < CN:
        gs.append((c, c + 1))
        c += 1
    return gs


def _build_nc():
    # Bacc (not plain Bass): its compile() runs generate_event_semaphores,
    # which splits multi-wait instructions onto InstEventSemaphore — TRN2
    # instructions can carry at most one sem wait.
    nc = bacc.Bacc(trn_type="TRN2")
    x = nc.dram_tensor("x", [BPC, S, D], mybir.dt.float32, kind="ExternalInput")
    tokens = nc.dram_tensor("tokens", [BPC, S], mybir.dt.int32, kind="ExternalInput")
    out = nc.dram_tensor("out", [BPC, D], mybir.dt.float32, kind="ExternalOutput")

    with TileKernel(nc) as tk:
        tk.body(x, tokens, out)
    nc.compile()
    return nc


class TileKernel:
    def __init__(self, nc):
        self.nc = nc
        self.ctx = ExitStack()
        self.tc = None

    def __enter__(self):
        self.tc = self.ctx.enter_context(tile.TileContext(self.nc))
        return self

    def __exit__(self, *exc):
        return self.ctx.__exit__(*exc)

    def body(self, x, tokens, out):
        nc = self.nc
        tc = self.tc
        ctx = self.ctx

        xpool = ctx.enter_context(tc.tile_pool(name="xpool", bufs=1))
        vpool = ctx.enter_context(tc.tile_pool(name="vpool", bufs=1))
        spool = ctx.enter_context(tc.tile_pool(name="spool", bufs=2))
        singles = ctx.enter_context(tc.tile_pool(name="singles", bufs=1))
        psum = ctx.enter_context(tc.tile_pool(name="psum", bufs=2, space="PSUM"))

        xeng = {"sync": nc.sync, "act": nc.scalar, "gpsimd": nc.gpsimd}[X_ENGINE]

        # DRAM views for the ragged layout.
        #   A: rows [0, PN*CN)   -> [PN, CN, D] (p-major, 33 rows contiguous)
        #   C: rows [PN*CN, S)   -> [4, 1, D]
        xA = [
            x[b, : PN * CN, :].rearrange("(p c) d -> p c d", c=CN)
            for b in range(BPC)
        ]
        xC = [
            x[b, PN * CN :, :].rearrange("(p c) d -> p c d", c=CS)
            for b in range(BPC)
        ]
        tA = [
            tokens[b, : PN * CN].rearrange("(p c) -> p c", c=CN)
            for b in range(BPC)
        ]
        tC = [
            tokens[b, PN * CN :].rearrange("(p c) -> p c", c=CS)
            for b in range(BPC)
        ]
        oa = out[:].rearrange("b d -> (b d)")  # [BPC*512]

        # --- x streams: static full-batch tiles, every DMA writes its own
        # region exactly once (no ring, no WAR hazards).  float32r: the DMA
        # is a pure bit copy; single-pass fp32 matmul (4x faster than fp32's
        # two half-rate passes); mask weights are exact 0/1 and PSUM still
        # accumulates in fp32.  Partitions 124..127, chunks >= 1 are never
        # written NOR read (K=124 matmuls), so no memset padding is needed.
        xb = [
            xpool.tile([P, CN, D], mybir.dt.float32r, name=f"xb{b}")
            for b in range(BPC)
        ]
        groups = _a_groups()
        for b in range(BPC):
            # The tiny slow-partition load rides the SP ring, keeping the ACT
            # ring a pure stream of big A transfers.
            nc.sync.dma_start(
                out=xb[b][PN:, 0:CS, :], in_=xC[b].bitcast(mybir.dt.float32r)
            )
            for (c0, c1) in groups:
                xeng.dma_start(
                    out=xb[b][:PN, c0:c1, :],
                    in_=xA[b][:, c0:c1, :].bitcast(mybir.dt.float32r),
                )

        # --- mask + counts ---------------------------------------------------
        # tok is memset to 0 first so the unwritten pad (partitions 124..127,
        # chunks >= 1) yields valid == 0 and a correct rowsum.
        tok = vpool.tile([P, BPC, CN], mybir.dt.int32)
        nc.vector.memset(tok, 0)
        for b in range(BPC):
            nc.sync.dma_start(out=tok[:PN, b, :], in_=tA[b])
            nc.sync.dma_start(out=tok[PN:, b, 0:CS], in_=tC[b])
        valid = vpool.tile([P, BPC, CN], mybir.dt.float32r)
        nc.vector.tensor_scalar(
            out=valid, in0=tok, scalar1=0, scalar2=None,
            op0=mybir.AluOpType.not_equal,
        )
        rowsum = spool.tile([P, BPC], mybir.dt.float32)
        nc.vector.reduce_sum(
            out=rowsum, in_=valid.bitcast(mybir.dt.float32),
            axis=mybir.AxisListType.X,
        )

        ones = singles.tile([P, 1], mybir.dt.float32)
        nc.vector.memset(ones, 1.0)

        orow = [
            spool.tile([1, D], mybir.dt.float32, name=f"orow{b}")
            for b in range(BPC)
        ]

        for b in range(BPC):
            cnt = psum.tile([1, 1], mybir.dt.float32)
            nc.tensor.matmul(cnt, ones, rowsum[:, b:b + 1], start=True, stop=True)
            recip = spool.tile([1, 1], mybir.dt.float32)
            nc.vector.reciprocal(recip, cnt)

            # --- masked sum: chunk 0 contracts all 128 partitions, the rest
            # only the 124 normal ones.
            num = psum.tile([1, D], mybir.dt.float32)
            nc.tensor.matmul(
                num, valid[:, b, 0:1], xb[b][:, 0, :], start=True, stop=False
            )
            for c in range(1, CN):
                nc.tensor.matmul(
                    num, valid[:PN, b, c:c + 1], xb[b][:PN, c, :],
                    start=False, stop=(c == CN - 1),
                )

            # --- divide + store: b0's store overlaps b1's stream; only b1's
            # 2KB store sits on the tail.
            nc.vector.tensor_scalar_mul(orow[b], num, recip)
            nc.sync.dma_start(out=oa[b * D:(b + 1) * D], in_=orow[b])


def _get_nc():
    global _NC
    if _NC is None:
        _NC = _build_nc()
    return _NC


def _shard(x, tokens):
    x = np.ascontiguousarray(np.asarray(x, dtype=np.float32))
    tokens = np.ascontiguousarray(np.asarray(tokens, dtype=np.int32))
    return [
        {
            "x": x[c * BPC:(c + 1) * BPC],
            "tokens": tokens[c * BPC:(c + 1) * BPC],
        }
        for c in range(NCORES)
    ]


def kernel(x, tokens):
    res = run_bass_kernel_spmd(_get_nc(), _shard(x, tokens), core_ids=list(range(NCORES)))
    return np.concatenate([r["out"] for r in res.results], axis=0)


def _install_ntff_shim():
    """The agent image's antenv lacks axon_hooks, so bass_utils' trace path
    can't find the NTFF hook. Recreate the tiny get/set module and register
    trn_boot's ctypes-based hook against the injected libaxon_pjrt.so."""
    import sys
    import types

    if "antenv.axon_hooks" in sys.modules:
        return
    mod = types.ModuleType("antenv.axon_hooks")
    state = {"hook": None}
    mod.set_axon_ntff_profile_hook = lambda h: state.__setitem__("hook", h)
    mod.get_axon_ntff_profile_hook = lambda: state["hook"]
    sys.modules["antenv.axon_hooks"] = mod
    try:
        from trn_agent_boot.trn_boot import _ntff_profile_via_ctypes

        mod.set_axon_ntff_profile_hook(
            _ntff_profile_via_ctypes("/opt/axon/libaxon_pjrt.so")
        )
    except Exception:
        pass


def kernel_profiled(x, tokens):
    """Same as kernel() but with NTFF tracing; returns (out, BassKernelResults)."""
    _install_ntff_shim()
    res = run_bass_kernel_spmd(
        _get_nc(), _shard(x, tokens), core_ids=list(range(NCORES)), trace=True
    )
    out = np.concatenate([r["out"] for r in res.results], axis=0)
    return out, res


# revision 7
# speedup vs baseline: 2.7025x; 2.7025x over previous
"""Masked mean-pool (NonZeroAvgPool) Trainium2 Bass kernel.

out[b, d] = sum_s (tokens[b,s] != 0) * x[b,s,d] / sum_s (tokens[b,s] != 0)

Full shapes: x [16, 4096, 512] f32, tokens [16, 4096] i32 -> out [16, 512] f32.
Sharding: pure data parallel over batch; 2 batches per core on 8 cores.

Per-core program (shapes [2, 4096, 512] / [2, 4096] -> [2, 512]):
  - sequence rows are indexed s = p*32 + c  (p: SBUF partition, c: chunk)
    so every DMA is contiguous per partition.
  - valid[p, c] = (tokens != 0) as f32 via DVE not_equal
  - count      = ones[128,1].T @ rowsum(valid)        (PE, [1,1] PSUM)
  - num[1, D]  = sum_c valid[:, c].T @ x_tile[:, c, :] (PE, accumulated in PSUM)
  - out row    = num * (1/count)                       (DVE), then a 2KB store
    per batch (b0's store overlaps b1's stream).

x streams into two static full-batch SBUF tiles (no ring, no WAR hazards)
with TAPERED dma_start sizes per batch: (6,6,6,6,4,2,1,1) chunks.  The bulk
DMAs keep 12KB per-partition descriptors (near peak per-engine efficiency)
while the 1-chunk tail DMAs mean the last completion semaphore gates a
single matmul instead of a 16-chunk burst.  The PE trails the stream by
about one group, never idles >3.4us (HAM stays warm, matmuls run at
2.4GHz), and the post-last-byte critical path is 1 matmul + divide + 2KB
store.  DMA partition dim is always 128: the descriptor spray across the
16 SDMA engines splits the outer AP dim by its largest power-of-2 factor
(a 124-partition DMA collapses onto 4 engines - measured 3x slowdown).

The tile framework round-robins the 8 DMAHW completion-sem lanes with
automatic reuse waits; 18 DMAs total is fine (the v1 "8 DMA limit" was
wrong).
"""

import os
from contextlib import ExitStack

import numpy as np

import concourse.bacc as bacc
import concourse.bass as bass
import concourse.tile as tile
from concourse import mybir
from concourse.bass_utils import run_bass_kernel_spmd

B, S, D = 16, 4096, 512
NCORES = 8
BPC = B // NCORES  # batches per core = 2
P = 128            # SBUF partitions
CPB = S // P       # chunks per batch = 32

# Tapered chunks-per-dma_start. Bulk groups of 6: a 6-chunk DMA arrives in
# ~3.7us while its predecessor's matmuls take ~1.4us, so PE idle stays
# under the ~3.4us HAM re-throttle window.
GROUPS = [int(g) for g in os.environ.get("K_GROUPS", "6,6,6,6,4,2,1,1").split(",")]
assert sum(GROUPS) == CPB
X_ENGINE = os.environ.get("K_XENG", "act")  # sync | act | gpsimd

_NC = None


def _build_nc():
    # Bacc (not plain Bass): its compile() runs generate_event_semaphores,
    # which splits multi-wait instructions onto InstEventSemaphore — TRN2
    # instructions can carry at most one sem wait.
    nc = bacc.Bacc(trn_type="TRN2")
    x = nc.dram_tensor("x", [BPC, S, D], mybir.dt.float32, kind="ExternalInput")
    tokens = nc.dram_tensor("tokens", [BPC, S], mybir.dt.int32, kind="ExternalInput")
    out = nc.dram_tensor("out", [BPC, D], mybir.dt.float32, kind="ExternalOutput")

    # s = p*CPB + c : per-partition contiguous rows
    xa = x[:].rearrange("b (p c) d -> b p c d", p=P)   # [BPC, 128, 32, 512]
    ta = tokens[:].rearrange("b (p c) -> p b c", p=P)  # [128, BPC, 32]
    oa = out[:].rearrange("b d -> (b d)")              # [BPC*512]

    with TileKernel(nc) as tk:
        tk.body(xa, ta, oa)
    nc.compile()
    return nc


class TileKernel:
    def __init__(self, nc):
        self.nc = nc
        self.ctx = ExitStack()
        self.tc = None

    def __enter__(self):
        self.tc = self.ctx.enter_context(tile.TileContext(self.nc))
        return self

    def __exit__(self, *exc):
        return self.ctx.__exit__(*exc)

    def body(self, xa, ta, oa):
        nc = self.nc
        tc = self.tc
        ctx = self.ctx

        xpool = ctx.enter_context(tc.tile_pool(name="xpool", bufs=1))
        vpool = ctx.enter_context(tc.tile_pool(name="vpool", bufs=1))
        spool = ctx.enter_context(tc.tile_pool(name="spool", bufs=2))
        singles = ctx.enter_context(tc.tile_pool(name="singles", bufs=1))
        psum = ctx.enter_context(tc.tile_pool(name="psum", bufs=2, space="PSUM"))

        xeng = {"sync": nc.sync, "act": nc.scalar, "gpsimd": nc.gpsimd}[X_ENGINE]

        # --- x streams first: every DMA writes its own region of a static
        # full-batch tile exactly once.  float32r: the DMA is a pure bit
        # copy; single-pass fp32 matmul (4x faster than fp32's two half-rate
        # passes); mask weights are exact 0/1, PSUM accumulates in fp32.
        xb = [
            xpool.tile([P, CPB, D], mybir.dt.float32r, name=f"xb{b}")
            for b in range(BPC)
        ]
        for b in range(BPC):
            c0 = 0
            for g in GROUPS:
                xeng.dma_start(
                    out=xb[b][:, c0:c0 + g, :],
                    in_=xa[b, :, c0:c0 + g, :].bitcast(mybir.dt.float32r),
                )
                c0 += g

        # --- mask + counts for both batches (one tok DMA) --------------------
        tok = vpool.tile([P, BPC, CPB], mybir.dt.int32)
        nc.sync.dma_start(out=tok, in_=ta)
        # valid is declared float32r so the fp32r matmul's verifier sees a
        # rounded producer; its values (0.0/1.0) are exact in any precision.
        valid = vpool.tile([P, BPC, CPB], mybir.dt.float32r)
        nc.vector.tensor_scalar(
            out=valid, in0=tok, scalar1=0, scalar2=None,
            op0=mybir.AluOpType.not_equal,
        )
        rowsum = spool.tile([P, BPC], mybir.dt.float32)
        nc.vector.reduce_sum(
            out=rowsum, in_=valid.bitcast(mybir.dt.float32),
            axis=mybir.AxisListType.X,
        )

        ones = singles.tile([P, 1], mybir.dt.float32)
        nc.vector.memset(ones, 1.0)

        orow = [
            spool.tile([1, D], mybir.dt.float32, name=f"orow{b}")
            for b in range(BPC)
        ]

        for b in range(BPC):
            cnt = psum.tile([1, 1], mybir.dt.float32)
            nc.tensor.matmul(cnt, ones, rowsum[:, b:b + 1], start=True, stop=True)
            recip = spool.tile([1, 1], mybir.dt.float32)
            nc.vector.reciprocal(recip, cnt)

            # --- masked sum: one matmul per chunk, gated by its group's DMA.
            num = psum.tile([1, D], mybir.dt.float32)
            for c in range(CPB):
                nc.tensor.matmul(
                    num, valid[:, b, c:c + 1], xb[b][:, c, :],
                    start=(c == 0), stop=(c == CPB - 1),
                )

            # --- divide + store: b0's store overlaps b1's stream; only b1's
            # 2KB store sits on the tail.
            nc.vector.tensor_scalar_mul(orow[b], num, recip)
            nc.sync.dma_start(out=oa[b * D:(b + 1) * D], in_=orow[b])


def _get_nc():
    global _NC
    if _NC is None:
        _NC = _build_nc()
    return _NC


def _shard(x, tokens):
    x = np.ascontiguousarray(np.asarray(x, dtype=np.float32))
    tokens = np.ascontiguousarray(np.asarray(tokens, dtype=np.int32))
    return [
        {
            "x": x[c * BPC:(c + 1) * BPC],
            "tokens": tokens[c * BPC:(c + 1) * BPC],
        }
        for c in range(NCORES)
    ]


def kernel(x, tokens):
    res = run_bass_kernel_spmd(_get_nc(), _shard(x, tokens), core_ids=list(range(NCORES)))
    return np.concatenate([r["out"] for r in res.results], axis=0)


def _install_ntff_shim():
    """The agent image's antenv lacks axon_hooks, so bass_utils' trace path
    can't find the NTFF hook. Recreate the tiny get/set module and register
    trn_boot's ctypes-based hook against the injected libaxon_pjrt.so."""
    import sys
    import types

    if "antenv.axon_hooks" in sys.modules:
        return
    mod = types.ModuleType("antenv.axon_hooks")
    state = {"hook": None}
    mod.set_axon_ntff_profile_hook = lambda h: state.__setitem__("hook", h)
    mod.get_axon_ntff_profile_hook = lambda: state["hook"]
    sys.modules["antenv.axon_hooks"] = mod
    try:
        from trn_agent_boot.trn_boot import _ntff_profile_via_ctypes

        mod.set_axon_ntff_profile_hook(
            _ntff_profile_via_ctypes("/opt/axon/libaxon_pjrt.so")
        )
    except Exception:
        pass


def kernel_profiled(x, tokens):
    """Same as kernel() but with NTFF tracing; returns (out, BassKernelResults)."""
    _install_ntff_shim()
    res = run_bass_kernel_spmd(
        _get_nc(), _shard(x, tokens), core_ids=list(range(NCORES)), trace=True
    )
    out = np.concatenate([r["out"] for r in res.results], axis=0)
    return out, res


# revision 8
# speedup vs baseline: 2.8262x; 1.0458x over previous
"""Masked mean-pool (NonZeroAvgPool) Trainium2 Bass kernel.

out[b, d] = sum_s (tokens[b,s] != 0) * x[b,s,d] / sum_s (tokens[b,s] != 0)

Full shapes: x [16, 4096, 512] f32, tokens [16, 4096] i32 -> out [16, 512] f32.
Sharding: pure data parallel over batch; 2 batches per core on 8 cores.

Per-core program (shapes [2, 4096, 512] / [2, 4096] -> [2, 512]):
  - sequence rows are indexed s = p*32 + c  (p: SBUF partition, c: chunk)
    so every DMA is contiguous per partition.
  - valid[p, c] = (tokens != 0) as f32 via DVE not_equal
  - count      = ones[128,1].T @ rowsum(valid)        (PE, [1,1] PSUM)
  - num[1, D]  = sum_c valid[:, c].T @ x_tile[:, c, :] (PE, accumulated in PSUM)
  - out row    = num * (1/count)                       (DVE), then a 2KB store
    per batch (b0's store overlaps b1's stream).

x streams into two static full-batch SBUF tiles (no ring, no WAR hazards)
with TAPERED dma_start sizes per batch: (6,6,6,6,4,2,1,1) chunks.  The bulk
DMAs keep 12KB per-partition descriptors (near peak per-engine efficiency)
while the 1-chunk tail DMAs mean the last completion semaphore gates a
single matmul instead of a 16-chunk burst.  The PE trails the stream by
about one group, never idles >3.4us (HAM stays warm, matmuls run at
2.4GHz), and the post-last-byte critical path is 1 matmul + divide + 2KB
store.  DMA partition dim is always 128: the descriptor spray across the
16 SDMA engines splits the outer AP dim by its largest power-of-2 factor
(a 124-partition DMA collapses onto 4 engines - measured 3x slowdown).

The tile framework round-robins the 8 DMAHW completion-sem lanes with
automatic reuse waits; 18 DMAs total is fine (the v1 "8 DMA limit" was
wrong).
"""

import os
from contextlib import ExitStack

import numpy as np

import concourse.bacc as bacc
import concourse.bass as bass
import concourse.tile as tile
from concourse import mybir
from concourse.bass_utils import run_bass_kernel_spmd

B, S, D = 16, 4096, 512
NCORES = 8
BPC = B // NCORES  # batches per core = 2
P = 128            # SBUF partitions
CPB = S // P       # chunks per batch = 32

# Tapered chunks-per-dma_start. Bulk groups of 6: a 6-chunk DMA arrives in
# ~3.7us while its predecessor's matmuls take ~1.4us, so PE idle stays
# under the ~3.4us HAM re-throttle window.
GROUPS = [int(g) for g in os.environ.get("K_GROUPS", "6,6,6,6,4,2,1,1").split(",")]
assert sum(GROUPS) == CPB
X_ENGINE = os.environ.get("K_XENG", "act")  # sync | act | gpsimd

_NC = None


def _build_nc():
    # Bacc (not plain Bass): its compile() runs generate_event_semaphores,
    # which splits multi-wait instructions onto InstEventSemaphore — TRN2
    # instructions can carry at most one sem wait.
    nc = bacc.Bacc(trn_type="TRN2")
    x = nc.dram_tensor("x", [BPC, S, D], mybir.dt.float32, kind="ExternalInput")
    tokens = nc.dram_tensor("tokens", [BPC, S], mybir.dt.int32, kind="ExternalInput")
    out = nc.dram_tensor("out", [BPC, D], mybir.dt.float32, kind="ExternalOutput")

    # s = p*CPB + c : per-partition contiguous rows
    xa = x[:].rearrange("b (p c) d -> b p c d", p=P)   # [BPC, 128, 32, 512]
    ta = tokens[:].rearrange("b (p c) -> p b c", p=P)  # [128, BPC, 32]
    oa = out[:].rearrange("b d -> (b d)")              # [BPC*512]

    with TileKernel(nc) as tk:
        tk.body(xa, ta, oa)
    nc.compile()
    return nc


class TileKernel:
    def __init__(self, nc):
        self.nc = nc
        self.ctx = ExitStack()
        self.tc = None

    def __enter__(self):
        self.tc = self.ctx.enter_context(tile.TileContext(self.nc))
        return self

    def __exit__(self, *exc):
        return self.ctx.__exit__(*exc)

    def body(self, xa, ta, oa):
        nc = self.nc
        tc = self.tc
        ctx = self.ctx

        xpool = ctx.enter_context(tc.tile_pool(name="xpool", bufs=1))
        vpool = ctx.enter_context(tc.tile_pool(name="vpool", bufs=1))
        spool = ctx.enter_context(tc.tile_pool(name="spool", bufs=2))
        singles = ctx.enter_context(tc.tile_pool(name="singles", bufs=1))
        psum = ctx.enter_context(tc.tile_pool(name="psum", bufs=2, space="PSUM"))

        xeng = {"sync": nc.sync, "act": nc.scalar, "gpsimd": nc.gpsimd}[X_ENGINE]

        # --- x streams first: every DMA writes its own region of a static
        # full-batch tile exactly once.  float32r: the DMA is a pure bit
        # copy; single-pass fp32 matmul (4x faster than fp32's two half-rate
        # passes); mask weights are exact 0/1, PSUM accumulates in fp32.
        xb = [
            xpool.tile([P, CPB, D], mybir.dt.float32r, name=f"xb{b}")
            for b in range(BPC)
        ]
        for b in range(BPC):
            c0 = 0
            for g in GROUPS:
                xeng.dma_start(
                    out=xb[b][:, c0:c0 + g, :],
                    in_=xa[b, :, c0:c0 + g, :].bitcast(mybir.dt.float32r),
                )
                c0 += g

        # --- mask + counts for both batches (one tok DMA) --------------------
        tok = vpool.tile([P, BPC, CPB], mybir.dt.int32)
        nc.sync.dma_start(out=tok, in_=ta)
        # valid is declared float32r so the fp32r matmul's verifier sees a
        # rounded producer; its values (0.0/1.0) are exact in any precision.
        valid = vpool.tile([P, BPC, CPB], mybir.dt.float32r)
        nc.vector.tensor_scalar(
            out=valid, in0=tok, scalar1=0, scalar2=None,
            op0=mybir.AluOpType.not_equal,
        )
        rowsum = spool.tile([P, BPC], mybir.dt.float32)
        nc.vector.reduce_sum(
            out=rowsum, in_=valid.bitcast(mybir.dt.float32),
            axis=mybir.AxisListType.X,
        )

        ones = singles.tile([P, 1], mybir.dt.float32)
        nc.vector.memset(ones, 1.0)

        orow = [
            spool.tile([1, D], mybir.dt.float32, name=f"orow{b}")
            for b in range(BPC)
        ]

        for b in range(BPC):
            cnt = psum.tile([1, 1], mybir.dt.float32)
            nc.tensor.matmul(cnt, ones, rowsum[:, b:b + 1], start=True, stop=True)
            recip = spool.tile([1, 1], mybir.dt.float32)
            nc.vector.reciprocal(recip, cnt)

            # --- masked sum: one matmul per chunk, gated by its group's DMA.
            num = psum.tile([1, D], mybir.dt.float32)
            for c in range(CPB):
                nc.tensor.matmul(
                    num, valid[:, b, c:c + 1], xb[b][:, c, :],
                    start=(c == 0), stop=(c == CPB - 1),
                )

            # --- divide + store: b0's store overlaps b1's stream; only b1's
            # 2KB store sits on the tail.  The divide is split across DVE and
            # ACT so the two halves run in parallel (~0.45us instead of 0.74).
            h = D // 2
            nc.vector.tensor_scalar_mul(orow[b][:, :h], num[:, :h], recip)
            nc.scalar.mul(orow[b][:, h:], num[:, h:], recip)
            nc.sync.dma_start(out=oa[b * D:(b + 1) * D], in_=orow[b])


def _get_nc():
    global _NC
    if _NC is None:
        _NC = _build_nc()
    return _NC


def _shard(x, tokens):
    x = np.ascontiguousarray(np.asarray(x, dtype=np.float32))
    tokens = np.ascontiguousarray(np.asarray(tokens, dtype=np.int32))
    return [
        {
            "x": x[c * BPC:(c + 1) * BPC],
            "tokens": tokens[c * BPC:(c + 1) * BPC],
        }
        for c in range(NCORES)
    ]


def kernel(x, tokens):
    res = run_bass_kernel_spmd(_get_nc(), _shard(x, tokens), core_ids=list(range(NCORES)))
    return np.concatenate([r["out"] for r in res.results], axis=0)


def _install_ntff_shim():
    """The agent image's antenv lacks axon_hooks, so bass_utils' trace path
    can't find the NTFF hook. Recreate the tiny get/set module and register
    trn_boot's ctypes-based hook against the injected libaxon_pjrt.so."""
    import sys
    import types

    if "antenv.axon_hooks" in sys.modules:
        return
    mod = types.ModuleType("antenv.axon_hooks")
    state = {"hook": None}
    mod.set_axon_ntff_profile_hook = lambda h: state.__setitem__("hook", h)
    mod.get_axon_ntff_profile_hook = lambda: state["hook"]
    sys.modules["antenv.axon_hooks"] = mod
    try:
        from trn_agent_boot.trn_boot import _ntff_profile_via_ctypes

        mod.set_axon_ntff_profile_hook(
            _ntff_profile_via_ctypes("/opt/axon/libaxon_pjrt.so")
        )
    except Exception:
        pass


def kernel_profiled(x, tokens):
    """Same as kernel() but with NTFF tracing; returns (out, BassKernelResults)."""
    _install_ntff_shim()
    res = run_bass_kernel_spmd(
        _get_nc(), _shard(x, tokens), core_ids=list(range(NCORES)), trace=True
    )
    out = np.concatenate([r["out"] for r in res.results], axis=0)
    return out, res
